# revision 22
# baseline (speedup 1.0000x reference)
"""Causal self-attention with RoPE for Trainium2, sharded over 8 NeuronCores.

Sharding (Megatron-style): 8 cores = 4 batches x 2 head-groups (8 of 16
heads each). Each core: QKV column-slice projections [1024,512], RoPE,
causal attention for its 8 heads, and a row-slice output projection
producing a partial [2048,1024] (bf16). Host sums the two partials per
batch and adds bo.

Per-core kernel (Tile framework), fused per query-chunk (512 queries):

- Q/K projections run transposed (weight chunks as the stationary
  operand, x^T as the moving operand) so q^T/k^T land directly in
  [c, t] layout -- no PE transposes. Host permutes Wq/Wk columns within
  each head to [evens | odds] (cancels in the q.k dot product), which
  makes RoPE three full-width bf16 DVE ops: m1 = x*cos, u = x*sin_signed,
  rot = m1 + swap32(u), where swap32 (partition p <-> p^32) is done by
  4 small SBUF->SBUF DMAs issued from the idle GPSIMD queue.
- Scores S^T[k,q] = k^T.T @ q^T per head pair in PE row-groups 0/64
  (concurrent in the array). Diagonal blocks compute only the causally
  valid query suffix (N = 512-128m), which also shrinks exp and AV.
- exp on ACT (scale=1/sqrt(hd) folded in); the single lower-triangle
  [128,128] mask is applied only to the partial diagonal sub-block.
- AV: Y^T = V^T @ E for the two heads col-tiled at PSUM partitions 0/64
  (concurrent), accumulated over key blocks; a second all-ones [128,64]
  stationary matmul produces the softmax denominator replicated over all
  64 rows, so normalization is one reciprocal + one multiply covering
  both heads.
- Output projection consumes y^T directly; results DMA out as bf16.

No flash-attention running max is needed: scores are ~N(0, 0.17) and exp
cannot overflow; softmax(x) == softmax(x - max) exactly.
"""
import sys

if "/opt/trn_rl_repo" not in sys.path:
    sys.path.insert(0, "/opt/trn_rl_repo")

from contextlib import ExitStack

import numpy as np
import ml_dtypes

import concourse.bass as bass
import concourse.mybir as mybir
import concourse.tile as tile
from concourse import bacc
from concourse.masks import make_identity

bf16 = ml_dtypes.bfloat16

N_HEAD = 16
ROPE_BASE = 10000.0
B_FULL, T_FULL, C_FULL = 4, 2048, 1024
HD = 64
N_CORES = 8
QCW = 512  # query-chunk width
JBW = 128  # key-block width


def build_core_program(T=T_FULL, HL=8, C=C_FULL, has_bias=False, reps=1,
                       tuning=None):
    env = dict(tuning or {})
    env["T"], env["HL"], env["C"], env["has_bias"] = T, HL, C, has_bias
    env["CL"] = HL * HD
    env["NTB"] = T // 128
    env["NQC"] = T // QCW
    env["NCH"] = env["CL"] // 128
    env["KCH"] = C // 128
    env["NEH"] = C // 512

    f32 = mybir.dt.float32
    b16 = mybir.dt.bfloat16

    nc = bacc.Bacc("TRN2", target_bir_lowering=False, debug=False,
                   enable_asserts=False)

    env["xT"] = nc.dram_tensor("xT", [C, T], b16, kind="ExternalInput").ap()
    env["wq"] = nc.dram_tensor("wq", [C, env["CL"]], b16, kind="ExternalInput").ap()
    env["wk"] = nc.dram_tensor("wk", [C, env["CL"]], b16, kind="ExternalInput").ap()
    env["wv"] = nc.dram_tensor("wv", [C, env["CL"]], b16, kind="ExternalInput").ap()
    env["wo"] = nc.dram_tensor("wo", [env["CL"], C], b16, kind="ExternalInput").ap()
    env["cosd"] = nc.dram_tensor("cosct", [128, T], b16, kind="ExternalInput").ap()
    env["sind"] = nc.dram_tensor("sinct", [128, T], b16, kind="ExternalInput").ap()
    env["maskd"] = nc.dram_tensor("maskt", [JBW, JBW], b16,
                                  kind="ExternalInput").ap()
    env["o"] = nc.dram_tensor("o", [T, C], b16, kind="ExternalOutput").ap()
    names = ["xT", "wq", "wk", "wv", "wo", "cosct", "sinct", "maskt"]
    if has_bias:
        env["bqr"] = nc.dram_tensor("bqr", [1, env["CL"]], b16,
                                    kind="ExternalInput").ap()
        env["bkr"] = nc.dram_tensor("bkr", [1, env["CL"]], b16,
                                    kind="ExternalInput").ap()
        env["bvr"] = nc.dram_tensor("bvr", [1, env["CL"]], b16,
                                    kind="ExternalInput").ap()
        names += ["bqr", "bkr", "bvr"]

    with tile.TileContext(nc) as tc:
        with ExitStack() as ctx:
            _body(ctx, tc, env, reps)
    nc.compile()
    return nc, names


def _body(ctx, tc, env, reps):
    nc = tc.nc
    f32 = mybir.dt.float32
    b16 = mybir.dt.bfloat16
    T, HL, C = env["T"], env["HL"], env["C"]
    CL, NTB, NQC, NCH, KCH, NEH = (env["CL"], env["NTB"], env["NQC"],
                                   env["NCH"], env["KCH"], env["NEH"])
    has_bias = env["has_bias"]
    xT, wq, wk, wv, wo = env["xT"], env["wq"], env["wk"], env["wv"], env["wo"]
    cosd, sind, maskd, o = env["cosd"], env["sind"], env["maskd"], env["o"]

    const = ctx.enter_context(tc.tile_pool(name="const", bufs=1))
    persist = ctx.enter_context(tc.tile_pool(name="persist", bufs=1))
    work = ctx.enter_context(tc.tile_pool(name="work", bufs=1))

    # ---- constants / weights into SBUF (chunked DMAs -> parallel queues)
    xT_sb = const.tile([128, KCH, T], b16)
    wq_sb = const.tile([128, KCH, CL], b16)
    wk_sb = const.tile([128, KCH, CL], b16)
    wv_sb = const.tile([128, KCH, CL], b16)
    for kc in range(KCH):
        sl = slice(kc * 128, (kc + 1) * 128)
        nc.sync.dma_start(out=xT_sb[:, kc, :], in_=xT[sl, :])
        nc.sync.dma_start(out=wq_sb[:, kc, :], in_=wq[sl, :])
        nc.sync.dma_start(out=wk_sb[:, kc, :], in_=wk[sl, :])
        nc.sync.dma_start(out=wv_sb[:, kc, :], in_=wv[sl, :])
    wo_sb = const.tile([128, NCH, C], b16)
    for cc in range(NCH):
        nc.sync.dma_start(out=wo_sb[:, cc, :],
                          in_=wo[cc * 128:(cc + 1) * 128, :])
    cos_sb = const.tile([128, T], b16)
    nc.sync.dma_start(out=cos_sb, in_=cosd)
    sin_sb = const.tile([128, T], b16)
    nc.sync.dma_start(out=sin_sb, in_=sind)
    mask_sb = const.tile([128, JBW], b16)
    nc.sync.dma_start(out=mask_sb, in_=maskd)
    ones64 = const.tile([128, 64], b16)
    nc.vector.memset(ones64, 1.0)
    ident = const.tile([128, 128], b16)
    make_identity(nc, ident)
    if has_bias:
        onesrow = const.tile([1, QCW], b16)
        nc.vector.memset(onesrow, 1.0)
        brows = {}
        for which in ("q", "k", "v"):
            t = const.tile([1, CL], b16, tag=f"b{which}")
            nc.sync.dma_start(out=t, in_=env[f"b{which}r"])
            brows[which] = t

    psum = ctx.enter_context(tc.tile_pool(name="ps", bufs=1, space="PSUM"))

    pjbufs = env.get("pjbufs", 2)
    sbufs = env.get("sbufs", 2)

    npair = max(NCH // 2, 1)
    pw = 2 if NCH >= 2 else 1

    def qk_pair_produce(qc, which, cp, qT_t, kT_sb):
        """Projection chains for one chunk-pair + PSUM->SBUF copy + swap-DMA
        issue. Returns state for qk_pair_finish."""
        ts = qc * QCW
        w_sb = wq_sb if which == "q" else wk_sb
        pss = []
        for ci in range(pw):
            cc = cp * pw + ci
            ps = psum.tile([128, QCW], f32, tag="pj", bufs=pjbufs,
                           name="ps_pj")
            for kc in range(KCH):
                nc.tensor.matmul(
                    ps, w_sb[:, kc, cc * 128:(cc + 1) * 128],
                    xT_sb[:, kc, ts:ts + QCW],
                    start=(kc == 0),
                    stop=(kc == KCH - 1 and not has_bias))
            if has_bias:
                nc.tensor.matmul(
                    ps, brows[which][0:1, cc * 128:(cc + 1) * 128],
                    onesrow, start=False, stop=True)
            pss.append(ps)
        x16 = work.tile([128, pw, QCW], b16, tag="x16", bufs=3)
        for ci in range(pw):
            nc.vector.tensor_copy(x16[:, ci, :], pss[ci])
        xsw = work.tile([128, pw, QCW], b16, tag="xsw", bufs=3)
        for blk in range(4):
            sp = blk ^ 1
            nc.sync.dma_start(
                out=xsw[blk * 32:(blk + 1) * 32, :, :],
                in_=x16[sp * 32:(sp + 1) * 32, :, :])
        return (qc, which, cp, qT_t, kT_sb, x16, xsw)

    def qk_pair_finish(st):
        qc, which, cp, qT_t, kT_sb, x16, xsw = st
        ts = qc * QCW
        cosb = cos_sb[:, ts:ts + QCW]
        cos2 = bass.AP(tensor=cosb.tensor, offset=cosb.offset,
                       ap=[cosb.ap[0], [0, pw], cosb.ap[1]])
        sinb = sin_sb[:, ts:ts + QCW]
        sin2 = bass.AP(tensor=sinb.tensor, offset=sinb.offset,
                       ap=[sinb.ap[0], [0, pw], sinb.ap[1]])
        m1 = work.tile([128, pw, QCW], b16, tag="m1", bufs=3)
        nc.vector.tensor_mul(m1, x16, cos2)
        us = work.tile([128, pw, QCW], b16, tag="us", bufs=3)
        nc.vector.tensor_mul(us, xsw, sin2)
        dstv = (qT_t[:, cp * pw:(cp + 1) * pw, :] if which == "q"
                else kT_sb[:, cp * pw:(cp + 1) * pw, ts:ts + QCW])
        nc.vector.tensor_add(dstv, m1, us)

    def qk_proj_rope(qc, qT_t, kT_sb):
        """All chunk-pairs, software-pipelined one pair deep so the swap-DMA
        latency hides behind the next pair's chains + copy."""
        pend = None
        for which in ("q", "k"):
            for cp in range(npair):
                st = qk_pair_produce(qc, which, cp, qT_t, kT_sb)
                if pend is not None:
                    qk_pair_finish(pend)
                pend = st
        qk_pair_finish(pend)

    def v_proj(tb, v_sb):
        ps = psum.tile([128, CL], f32, tag="pj", bufs=pjbufs, name="ps_pj")
        for kc in range(KCH):
            nc.tensor.matmul(ps, xT_sb[:, kc, tb * 128:(tb + 1) * 128],
                             wv_sb[:, kc, :], start=(kc == 0),
                             stop=(kc == KCH - 1 and not has_bias))
        if has_bias:
            nc.tensor.matmul(ps, onesrow[:, 0:128], brows["v"],
                             start=False, stop=True)
        nc.vector.tensor_copy(v_sb[:, tb, :, :], ps)

    def attention_g(qc, g, qT_t, yT_t, kT_sb, v_sb):
        qs = qc * QCW
        njb = (qs + QCW) // JBW
        if True:
            ps_av = psum.tile([128, QCW], f32, tag="av", bufs=1, name="ps_av")
            ps_d = psum.tile([128, QCW], f32, tag="d", bufs=1, name="ps_d")
            for jb in range(njb):
                m = jb - (njb - 4)  # >= 0 on diagonal blocks
                off = max(m, 0) * JBW
                ps_s = psum.tile([128, 2 * QCW], f32, tag="s", bufs=sbufs,
                                 name="ps_s")
                for hh in range(2):
                    base = hh * 64
                    nc.tensor.matmul(
                        ps_s[:, hh * QCW + off:(hh + 1) * QCW],
                        kT_sb[base:base + 64, g, jb * JBW:(jb + 1) * JBW],
                        qT_t[base:base + 64, g, off:QCW],
                        start=True, stop=(m < 0))
                if m >= 0:
                    # causal mask: add -240 to the invalid entries of the
                    # partial diagonal sub-block (identity-stationary matmul
                    # adds an arbitrary constant matrix); exp then yields ~0
                    for hh in range(2):
                        nc.tensor.matmul(
                            ps_s[:, hh * QCW + off:hh * QCW + off + JBW],
                            ident, mask_sb, start=False, stop=True)
                e = work.tile([128, 2, QCW], b16, tag="e", bufs=4)
                s3 = ps_s.rearrange("p (two q) -> p two q", two=2)
                nc.scalar.activation(
                    out=e[:, :, off:], in_=s3[:, :, off:],
                    func=mybir.ActivationFunctionType.Exp,
                    scale=float(1.0 / np.sqrt(HD)))
                for hh in range(2):
                    h = g * 2 + hh
                    nc.tensor.matmul(
                        ps_av[hh * 64:(hh + 1) * 64, off:],
                        v_sb[:, jb, h, :], e[:, hh, off:],
                        start=(jb == 0), stop=(jb == njb - 1),
                        skip_group_check=(hh == 1))
                for hh in range(2):
                    nc.tensor.matmul(
                        ps_d[hh * 64:(hh + 1) * 64, off:],
                        ones64, e[:, hh, off:],
                        start=(jb == 0), stop=(jb == njb - 1),
                        skip_group_check=(hh == 1))
            rinv = work.tile([128, QCW], f32, tag="rinv", bufs=2)
            nc.vector.reciprocal_approx_fast(out=rinv, in_=ps_d)
            nc.vector.tensor_mul(yT_t[:, g, :], ps_av, rinv)

    def out_proj(qc, yT_t):
        for t4 in range(4):
            tb = qc * 4 + t4
            for eh in range(NEH):
                ps_o = psum.tile([128, 512], f32, tag="pj", bufs=pjbufs,
                                 name="ps_pj")
                for cc in range(NCH):
                    nc.tensor.matmul(ps_o,
                                     yT_t[:, cc, t4 * 128:(t4 + 1) * 128],
                                     wo_sb[:, cc, eh * 512:(eh + 1) * 512],
                                     start=(cc == 0), stop=(cc == NCH - 1))
                osb = work.tile([128, 512], b16, tag="osb", bufs=3)
                nc.vector.tensor_copy(osb, ps_o)
                nc.sync.dma_start(
                    out=o[tb * 128:(tb + 1) * 128, eh * 512:(eh + 1) * 512],
                    in_=osb)

    import os
    abl = os.environ.get("KABL", "")

    def body_once():
        kT_sb = persist.tile([128, NCH, T], b16, tag="kT", bufs=2)
        v_sb = persist.tile([128, NTB, HL, 64], b16, tag="v", bufs=2)
        qT_t = work.tile([128, NCH, QCW], b16, tag="qT", bufs=2)
        if "noproj" in abl:
            nc.gpsimd.memset(kT_sb, 0.01)
            nc.gpsimd.memset(v_sb, 0.01)
            nc.gpsimd.memset(qT_t, 0.01)
        else:
            qk_proj_rope(0, qT_t, kT_sb)
            for tb in range(4):
                v_proj(tb, v_sb)
        for qc in range(NQC):
            yT_t = work.tile([128, NCH, QCW], b16, tag="yT", bufs=2)
            if "noattn" in abl:
                nc.gpsimd.memset(yT_t, 0.01)
            else:
                for g in range(NCH):
                    attention_g(qc, g, qT_t, yT_t, kT_sb, v_sb)
            if qc + 1 < NQC and "noproj" not in abl:
                qT_t = work.tile([128, NCH, QCW], b16, tag="qT", bufs=2)
                qk_proj_rope(qc + 1, qT_t, kT_sb)
                for tb in range((qc + 1) * 4, (qc + 2) * 4):
                    v_proj(tb, v_sb)
            if "noout" not in abl:
                out_proj(qc, yT_t)

    nbody = 2 if "body2" in abl else 1
    if reps == 1:
        for _ in range(nbody):
            body_once()
    else:
        with tc.For_i(0, reps, 1):
            for _ in range(nbody):
                body_once()


def _qk_perm(HL):
    """Column permutation putting each head's dims in [evens | odds] order."""
    p = []
    for h in range(HL):
        p.extend(h * HD + np.arange(0, HD, 2))
        p.extend(h * HD + np.arange(1, HD, 2))
    return np.asarray(p)


def make_host_aux(T=T_FULL):
    """cos/sin caches [128, T] bf16 (RoPE in [c, t] layout with the ev/od
    split and sign folded into sin) and the [128, 128] lower-triangle mask."""
    inv_freq = (1.0 / ROPE_BASE ** (np.arange(0, HD, 2, dtype=np.float32)
                                    / np.float32(HD))).astype(np.float32)
    pos = np.arange(T, dtype=np.float32)
    p = np.arange(128)
    freqs = np.outer(inv_freq[p % 32], pos)  # [128, T]
    cos = np.cos(freqs).astype(bf16)
    sgn = np.where((p % 64) < 32, -1.0, 1.0).astype(np.float32)
    sin = (np.sin(freqs) * sgn[:, None]).astype(bf16)
    kk = np.arange(JBW)[:, None]
    qq = np.arange(JBW)[None, :]
    mask = np.where(qq >= kk, 0.0, -240.0).astype(bf16)
    return cos, sin, mask


def make_in_maps(x, Wq, bq, Wk, bk, Wv, bv, Wo, T=T_FULL, HL=8):
    """Shard inputs for the 8 cores: core i = (batch i//2, head-group i%2)."""
    CL = HL * HD
    cos, sin, mask = make_host_aux(T)
    perm = _qk_perm(HL)
    B = x.shape[0]
    n_groups = N_CORES // B
    has_bias = bool(np.any(bq) or np.any(bk) or np.any(bv))
    in_maps = []
    for core in range(N_CORES):
        b, g = divmod(core, n_groups)
        cols = slice(g * CL, (g + 1) * CL)
        m = {
            "xT": np.ascontiguousarray(x[b].astype(bf16).T),
            "wq": np.ascontiguousarray(Wq[:, cols][:, perm].astype(bf16)),
            "wk": np.ascontiguousarray(Wk[:, cols][:, perm].astype(bf16)),
            "wv": np.ascontiguousarray(Wv[:, cols].astype(bf16)),
            "wo": np.ascontiguousarray(Wo[cols, :].astype(bf16)),
            "cosct": cos, "sinct": sin, "maskt": mask,
        }
        if has_bias:
            m["bqr"] = bq[cols][perm][None, :].astype(bf16)
            m["bkr"] = bk[cols][perm][None, :].astype(bf16)
            m["bvr"] = bv[None, cols].astype(bf16)
        in_maps.append(m)
    return in_maps, has_bias


_CACHE = {}


def kernel(x, Wq, bq, Wk, bk, Wv, bv, Wo, bo):
    x = np.asarray(x, np.float32)
    B, T, C = x.shape
    assert (B, T, C) == (B_FULL, T_FULL, C_FULL), (B, T, C)
    in_maps, has_bias = make_in_maps(x, Wq, bq, Wk, bk, Wv, bv, Wo)
    key = ("full", has_bias)
    if key not in _CACHE:
        _CACHE[key] = build_core_program(T=T_FULL, HL=8, C=C_FULL,
                                         has_bias=has_bias)
    nc, _names = _CACHE[key]
    from concourse.bass_utils import run_bass_kernel_spmd
    res = run_bass_kernel_spmd(nc, in_maps, core_ids=list(range(N_CORES)),
                               trace=False)
    bo32 = np.asarray(bo, np.float32)
    out = np.empty((B, T, C), np.float32)
    n_groups = N_CORES // B
    for b in range(B):
        acc = res.results[b * n_groups]["o"].astype(np.float32)
        for g in range(1, n_groups):
            acc = acc + res.results[b * n_groups + g]["o"].astype(np.float32)
        out[b] = acc + bo32[None, :]
    return out
